# revision 13
# baseline (speedup 1.0000x reference)
"""Bass/Trainium2 kernel for nn_DimeNet_22737556865501.

Strategy (v2)
-------------
Same circulant-structure collapse as v1 (per-atom dense math on the 16
local displacement vectors), but restructured around three observations:

1. Normalize first: with Vhat = V/|V|, the Gram matrix of Vhat IS cos(alpha),
   eliminating the ab/amg/den/ln/exp chain entirely.
2. Half-angle: alpha = 2*atan(sqrt((1-c)/(1+c))) and
   (1-c)/(1+c) = 2/(1+c) - 1, so alpha costs exactly three ACT ops
   (reciprocal with bias, sqrt with scale/bias, arctan) after one clamp.
   The factor 2 folds into env2 = 2*env (already needed).
3. bf16 + DVE 2x packing for the two dense blocks (Gram pair products and
   the alpha@erbf contraction), with the c-dimension padded to 4 so the
   innermost axis stays even/step-1; reductions use DVE tensor_reduce or
   short trees with f32 final accumulation.

Activation tables used (one load each, phases globally ordered):
  reciprocal_sqrt_and_small (Square, Rsqrt) -> reciprocal_and_small
  (Reciprocal) -> sqrt_and_others (Sqrt) -> trig_and_small (Sin, Arctan).

Sharding: atoms partitioned across the 8 NeuronCores (4096 each); each core
writes its own 65536x6 output rows; host concatenates. Host verifies the
circulant graph and falls back to exact numpy otherwise.
"""

import numpy as np

N_ATOMS = 32768
DEG = 16
HALF = DEG // 2
N_CORES = 8
J_PER_CORE = N_ATOMS // N_CORES  # 4096
P = 128  # partitions / atoms per tile
N_TILES = J_PER_CORE // P  # 32
WIN_ROWS = J_PER_CORE + DEG  # 4112 (8-row halo each side)
N_RBF = 6
CUTOFF = 5.0
ENV_P = 6
A_ = -(ENV_P + 1) * (ENV_P + 2) / 2.0  # -28
B_ = float(ENV_P * (ENV_P + 2))  # 48
C_ = -ENV_P * (ENV_P + 1) / 2.0  # -21
EA = 2.0 * A_  # -56
EB = 2.0 * B_  # 96
EC = 2.0 * C_  # -42
TWO_PI = float(2.0 * np.pi)

# tile ownership: t % 8 < GD_DVE -> Gram on DVE; t % 8 < CD_DVE -> contraction
# on DVE (rest on Pool/GpSimd)
GD_DVE = 5
CD_DVE = 7

OFFS = np.concatenate([np.arange(1, HALF + 1), -np.arange(1, HALF + 1)])

_cached_nc = None


def _expected_graph():
    half = HALF
    offs = np.concatenate([np.arange(1, half + 1), N_ATOMS - np.arange(1, half + 1)])
    j = np.arange(N_ATOMS)
    nbr_dst = (j[:, None] + offs[None, :]) % N_ATOMS
    nbr_list = np.stack([np.repeat(j, DEG), nbr_dst.reshape(-1)], 1)
    o1, o2 = np.meshgrid(offs, offs, indexing="ij")
    keep = o1 != o2
    o1, o2 = o1[keep], o2[keep]
    i = (j[:, None] + o1[None, :]) % N_ATOMS
    k = (j[:, None] + o2[None, :]) % N_ATOMS
    jc = np.broadcast_to(j[:, None], i.shape)
    angle_list = np.stack([i.reshape(-1), jc.reshape(-1), k.reshape(-1)], 1)
    return nbr_list.astype(np.int64), angle_list.astype(np.int64)


def _graph_matches(nbr_list, angle_list):
    if nbr_list.shape != (N_ATOMS * DEG, 2):
        return False
    if angle_list.shape != (N_ATOMS * DEG * (DEG - 1), 3):
        return False
    exp_nbr, exp_ang = _expected_graph()
    return np.array_equal(np.asarray(nbr_list), exp_nbr) and np.array_equal(
        np.asarray(angle_list), exp_ang
    )


def _fallback_numpy(xyz, nbr_list, angle_list):
    """Exact numpy mirror of the jax reference (general graph)."""
    xyz = np.asarray(xyz, dtype=np.float32)
    nbr = np.asarray(nbr_list)
    ang = np.asarray(angle_list)
    E = nbr.shape[0]
    r_ji = xyz[ang[:, 0]] - xyz[ang[:, 1]]
    r_jk = xyz[ang[:, 2]] - xyz[ang[:, 1]]
    dot = np.sum(r_ji * r_jk, axis=-1)
    crs = np.linalg.norm(np.cross(r_ji, r_jk), axis=-1)
    alpha = np.arctan2(crs, dot)
    diff = xyz[nbr[:, 0]] - xyz[nbr[:, 1]]
    d = np.linalg.norm(diff, axis=-1)
    n = np.arange(1, N_RBF + 1, dtype=xyz.dtype)
    dc = (d / CUTOFF)[:, None]
    env = 1.0 / dc + A_ * dc ** (ENV_P - 1) + B_ * dc**ENV_P + C_ * dc ** (ENV_P + 1)
    e_rbf = env * np.sin(n * np.pi * dc)
    keys = nbr[:, 0] * N_ATOMS + nbr[:, 1]
    order = np.argsort(keys, kind="stable")
    ji_idx = order[np.searchsorted(keys[order], ang[:, 1] * N_ATOMS + ang[:, 0])]
    kj_idx = order[np.searchsorted(keys[order], ang[:, 2] * N_ATOMS + ang[:, 1])]
    trip = alpha[:, None] * e_rbf[kj_idx]
    out = np.zeros((E, N_RBF), dtype=np.float32)
    np.add.at(out, ji_idx, trip.astype(np.float32))
    return out


# ---------------------------------------------------------------------------
# Device kernel
# ---------------------------------------------------------------------------


def _build_device_kernel():
    import concourse.bacc as bacc
    import concourse.mybir as mybir
    from concourse.bass_types import AP
    from concourse.tile import TileContext

    F32 = mybir.dt.float32
    BF16 = mybir.dt.bfloat16
    F16 = mybir.dt.float16
    I32 = mybir.dt.int32
    I16 = mybir.dt.int16
    ALU = mybir.AluOpType
    ACT = mybir.ActivationFunctionType
    AX = mybir.AxisListType

    # Steer the activation-table-load pass so each function resolves to one
    # set and the phase ordering needs exactly four table loads.
    from concourse.hw_specs import get_activation_tables

    assign = {
        ACT.Square: "natural_log_exp_and_others",
        ACT.Ln: "natural_log_exp_and_others",
        ACT.Exp: "natural_log_exp_and_others",
        ACT.Sqrt: "sqrt_and_others",
        ACT.Sin: "trig_and_small",
        ACT.Arctan: "trig_and_small",
    }
    tabs = get_activation_tables("gen3")
    for name, fns in tabs.items():
        for fn, keep in assign.items():
            if name != keep:
                fns.discard(fn)

    def sub(base: AP, off: int, dims) -> AP:
        """Sub-AP of an SBUF tile: keep partition dim, custom free dims."""
        return AP(
            tensor=base.tensor,
            offset=base.offset + off,
            ap=[list(base.ap[0]), *[list(d) for d in dims]],
        )

    T = N_TILES  # 32
    CH = 4  # pipeline chunks
    TC = T // CH  # tiles per chunk
    nc = bacc.Bacc("TRN2", target_bir_lowering=False, debug=False, num_devices=N_CORES)
    win = nc.dram_tensor("win", [WIN_ROWS, 3], F32, kind="ExternalInput")
    consts = nc.dram_tensor("consts", [P, 16], F32, kind="ExternalInput")
    out = nc.dram_tensor("out", [J_PER_CORE * DEG, N_RBF], F32, kind="ExternalOutput")
    al2d = nc.dram_tensor("al2d", [J_PER_CORE, 256], F16, kind="Internal")
    erbd = nc.dram_tensor("erbd", [J_PER_CORE, 96], F16, kind="Internal")

    with TileContext(nc) as tc:
        with (
            tc.tile_pool(name="big", bufs=1) as big,
            tc.tile_pool(name="work", bufs=2) as work,
            tc.psum_pool(name="pspool", bufs=1) as pspool,
        ):
            nco = big.tile([P, 16], F32, name="nco")
            nc.sync.dma_start(nco[:], consts[:])
            # register -1.0 (held in consts slot 7) for activation bias use
            nc.const_aps.aps[(F32, -1.0)] = sub(nco[:], 7, [[1, 1]])

            # ---- global buffers (per-partition free sizes) ----
            w = big.tile([P, T * 51], F32, name="w")  # window
            v = big.tile([P, T * 48], F32, name="v")  # V f32 [t,b,3]
            n2 = big.tile([P, T * 16], F32, name="n2")
            yn = big.tile([P, T * 16], F32, name="yn")  # 1/d
            dd = big.tile([P, T * 16], F32, name="dd")  # d
            vh = big.tile([P, T * 48], F32, name="vh")  # Vhat f32 [t,b,3]
            dc = big.tile([P, T * 16], F32, name="dc")
            q = big.tile([P, T * 16], F32, name="q")
            x5 = big.tile([P, T * 16], F32, name="x5")
            h1 = big.tile([P, T * 16], F32, name="h1")
            env2 = big.tile([P, T * 16], F32, name="env2")
            sa2 = big.tile([P, T * 96], F32, name="sa2")  # [t,r,b] turns
            ki = big.tile([P, T * 96], I16, name="ki")
            kf = big.tile([P, T * 96], F32, name="kf")
            sinv = big.tile([P, T * 96], F32, name="sinv")
            erbf = big.tile([P, T * 96], F16, name="erbf")  # [t,b,r]
            ch = big.tile([P, T * 256], F32, name="ch")  # cos alpha [t,b,a]
            rec = big.tile([P, T * 256], F32, name="rec")
            al2 = big.tile([P, T * 256], F16, name="al2")  # alpha/2 f16
            ot = big.tile([P, T * 96], F32, name="ot")  # out [t,a,r]

            def R(buf, c, per, dims):
                return sub(buf[:], c * TC * per, dims)

            # ---------- phase-major, chunk-minor pipeline ----------
            # window loads (one DMA per chunk)
            for c in range(CH):
                src = AP(
                    tensor=win,
                    offset=c * TC * P * 3,
                    ap=[[3, P], [P * 3, TC], [1, 51]],
                )
                nc.sync.dma_start(R(w, c, 51, [[1, TC * 51]]), src)

            # V[t,a,c]; a=0..7 <- +1..+8, a=8..15 <- -1..-8  (Pool)
            for c in range(CH):
                ctr = R(w, c, 51, [[51, TC], [0, 8], [1, 3]])
                ctr = AP(tensor=ctr.tensor, offset=ctr.offset + 24, ap=ctr.ap)
                nc.gpsimd.tensor_tensor(
                    R(v, c, 48, [[48, TC], [3, 8], [1, 3]]),
                    sub(R(w, c, 51, [[51, TC], [3, 8], [1, 3]]), 27, None)
                    if False
                    else AP(
                        tensor=w.tensor,
                        offset=w[:].offset + c * TC * 51 + 27,
                        ap=[list(w[:].ap[0]), [51, TC], [3, 8], [1, 3]],
                    ),
                    ctr,
                    ALU.subtract,
                )
                nc.gpsimd.tensor_tensor(
                    AP(
                        tensor=v.tensor,
                        offset=v[:].offset + c * TC * 48 + 24,
                        ap=[list(v[:].ap[0]), [48, TC], [3, 8], [1, 3]],
                    ),
                    AP(
                        tensor=w.tensor,
                        offset=w[:].offset + c * TC * 51 + 21,
                        ap=[list(w[:].ap[0]), [51, TC], [-3, 8], [1, 3]],
                    ),
                    ctr,
                    ALU.subtract,
                )

            # ---- norms: n2 = sum_c V^2 (Pool, 5 ops using q as scratch) ----
            for c in range(CH):
                vv0 = R(v, c, 48, [[48, TC], [3, 16], [0, 1]])
                vv1 = AP(tensor=v.tensor, offset=v[:].offset + c * TC * 48 + 1,
                         ap=[list(v[:].ap[0]), [48, TC], [3, 16], [0, 1]])
                vv2 = AP(tensor=v.tensor, offset=v[:].offset + c * TC * 48 + 2,
                         ap=[list(v[:].ap[0]), [48, TC], [3, 16], [0, 1]])
                n2c = R(n2, c, 16, [[16, TC], [1, 16], [0, 1]])
                qc = R(q, c, 16, [[16, TC], [1, 16], [0, 1]])
                nc.gpsimd.tensor_tensor(n2c, vv0, vv0, ALU.mult)
                nc.gpsimd.tensor_tensor(qc, vv1, vv1, ALU.mult)
                nc.gpsimd.tensor_tensor(n2c, n2c, qc, ALU.add)
                nc.gpsimd.tensor_tensor(qc, vv2, vv2, ALU.mult)
                nc.gpsimd.tensor_tensor(n2c, n2c, qc, ALU.add)

            # ---- 1/d via exp(-0.5 ln n2) (ACT, natural_log_exp set) ----
            for c in range(CH):
                nc.scalar.activation(
                    R(yn, c, 16, [[1, TC * 16]]), R(n2, c, 16, [[1, TC * 16]]),
                    ACT.Ln,
                )
                nc.scalar.activation(
                    R(yn, c, 16, [[1, TC * 16]]), R(yn, c, 16, [[1, TC * 16]]),
                    ACT.Exp, scale=-0.5,
                )

            for c in range(CH):
                # d = n2 * yn ; Vhat = V * yn (DVE)
                nc.vector.tensor_tensor(
                    R(dd, c, 16, [[1, TC * 16]]),
                    R(n2, c, 16, [[1, TC * 16]]),
                    R(yn, c, 16, [[1, TC * 16]]),
                    ALU.mult,
                )
                nc.vector.tensor_tensor(
                    R(vh, c, 48, [[48, TC], [3, 16], [1, 3]]),
                    R(v, c, 48, [[48, TC], [3, 16], [1, 3]]),
                    R(yn, c, 16, [[16, TC], [1, 16], [0, 3]]),
                    ALU.mult,
                )

            # ---- envelope: env2 = 2C*yn + dc^5*(EA + EB*dc + EC*dc^2) ----
            for c in range(CH):
                dcc = R(dc, c, 16, [[1, TC * 16]])
                qc = R(q, c, 16, [[1, TC * 16]])
                x5c = R(x5, c, 16, [[1, TC * 16]])
                h1c = R(h1, c, 16, [[1, TC * 16]])
                nc.vector.tensor_scalar(
                    dcc, R(dd, c, 16, [[1, TC * 16]]), 1.0 / CUTOFF, None, ALU.mult
                )
                nc.gpsimd.tensor_tensor(qc, dcc, dcc, ALU.mult)
                nc.gpsimd.tensor_tensor(x5c, qc, qc, ALU.mult)
                nc.gpsimd.tensor_tensor(x5c, x5c, dcc, ALU.mult)
                nc.vector.tensor_scalar(h1c, dcc, EC / 32.0, EB / 32.0, ALU.mult, ALU.add)
                nc.gpsimd.tensor_tensor(h1c, h1c, dcc, ALU.mult)
                nc.vector.tensor_scalar(h1c, h1c, EA / 32.0, None, ALU.add)
                nc.gpsimd.tensor_tensor(x5c, x5c, h1c, ALU.mult)
                nc.vector.scalar_tensor_tensor(
                    R(env2, c, 16, [[1, TC * 16]]),
                    R(yn, c, 16, [[1, TC * 16]]),
                    2.0 * CUTOFF / 32.0, x5c, ALU.mult, ALU.add,
                )

            # ---- sin args (turns) + range reduction ----
            for c in range(CH):
                nc.vector.tensor_tensor(
                    R(sa2, c, 96, [[96, TC], [16, 6], [1, 16]]),
                    R(dd, c, 16, [[16, TC], [0, 6], [1, 16]]),
                    sub(nco[:], 0, [[0, TC], [1, 6], [0, 16]]),
                    ALU.mult,
                )
                nc.vector.tensor_copy(
                    R(ki, c, 96, [[1, TC * 96]]), R(sa2, c, 96, [[1, TC * 96]])
                )
                nc.vector.tensor_copy(
                    R(kf, c, 96, [[1, TC * 96]]), R(ki, c, 96, [[1, TC * 96]])
                )
                nc.gpsimd.tensor_tensor(
                    R(sa2, c, 96, [[1, TC * 96]]),
                    R(sa2, c, 96, [[1, TC * 96]]),
                    R(kf, c, 96, [[1, TC * 96]]),
                    ALU.subtract,
                )

            # ---- Gram (Pool, f32): ch[t,b,a] = sum_c Vhat[b,c]*Vhat[a,c] ----
            for c in range(CH):
                for tl in range(TC):
                    t = c * TC + tl
                    p3 = work.tile([P, 768], F32, tag="p3", bufs=2)
                    nc.gpsimd.tensor_tensor(
                        sub(p3[:], 0, [[48, 16], [3, 16], [1, 3]]),
                        sub(vh[:], t * 48, [[0, 16], [3, 16], [1, 3]]),
                        sub(vh[:], t * 48, [[3, 16], [0, 16], [1, 3]]),
                        ALU.mult,
                    )
                    nc.gpsimd.tensor_tensor(
                        sub(ch[:], t * 256, [[1, 256]]),
                        sub(p3[:], 0, [[3, 256]]),
                        sub(p3[:], 1, [[3, 256]]),
                        ALU.add,
                    )
                    nc.gpsimd.tensor_tensor(
                        sub(ch[:], t * 256, [[1, 256]]),
                        sub(ch[:], t * 256, [[1, 256]]),
                        sub(p3[:], 2, [[3, 256]]),
                        ALU.add,
                    )

            # clamp so both ln args stay positive: |c| <= 1 - 2^-23
            for c in range(CH):
                nc.vector.tensor_scalar(
                    R(ch, c, 256, [[1, TC * 256]]),
                    R(ch, c, 256, [[1, TC * 256]]),
                    0.9999999, -0.9999999, ALU.min, ALU.max,
                )

            # ---- alpha/2 = atan(exp(0.5*(ln(1-c) - ln(1+c)))) ----
            for c in range(CH):
                cc = R(ch, c, 256, [[1, TC * 256]])
                rc = R(rec, c, 256, [[1, TC * 256]])
                nc.scalar.activation(rc, cc, ACT.Ln, bias=1.0)  # ln(1+c)
                nc.scalar.activation(cc, cc, ACT.Ln, bias=1.0, scale=-1.0)
                nc.vector.tensor_tensor(cc, cc, rc, ALU.subtract)
                nc.scalar.activation(rc, cc, ACT.Exp, scale=0.5)
            # sin goes through the same-set boundary; trig set loads once
            for c in range(CH):
                nc.scalar.activation(
                    R(al2, c, 256, [[1, TC * 256]]),
                    R(rec, c, 256, [[1, TC * 256]]),
                    ACT.Arctan,
                )
                nc.scalar.activation(
                    R(sinv, c, 96, [[1, TC * 96]]),
                    R(sa2, c, 96, [[1, TC * 96]]),
                    ACT.Sin, scale=TWO_PI,
                )
            # diagonal alpha := 0 (reference's i != k exclusion)
            for c in range(CH):
                dg = AP(
                    tensor=al2.tensor,
                    offset=al2[:].offset + c * TC * 256,
                    ap=[list(al2[:].ap[0]), [256, TC], [17, 16]],
                )
                nc.vector.tensor_scalar_mul(dg, dg, 0.0)

            # ---- e_rbf (x2/32): erbf[t,b,r] = env2[t,b] * sinv[t,r,b] ----
            for c in range(CH):
                nc.gpsimd.tensor_tensor(
                    R(erbf, c, 96, [[96, TC], [6, 16], [1, 6]]),
                    R(sinv, c, 96, [[96, TC], [1, 16], [16, 6]]),
                    R(env2, c, 16, [[16, TC], [1, 16], [0, 6]]),
                    ALU.mult,
                )

            # ---- contraction on PE: out[j,a,r] = sum_b al2[j,a,b]*erbf[j,b,r]
            # alpha (symmetric) becomes 8-atom block-diagonal stationary
            # weights via a DRAM bounce; erbf becomes the moving operand in
            # [(jsub,b), (g,r)] layout (uniform partition stride on the DRAM
            # side). PSUM accumulates f32; ACT drains with the x32 rescale.
            NG = J_PER_CORE // 8  # 512 groups of 8 atoms
            GPC = NG // CH  # 128 groups per pipeline chunk
            RND = 32  # groups per W2 scatter/matmul round
            w2 = big.tile([P, 2 * RND * P], F16, name="w2")  # double buffer
            erbB = big.tile([P, NG * 6], F16, name="erbB")
            nc.vector.memset(w2[:], 0.0)
            ps = pspool.tile([P, NG * 8], F32, name="ps")

            for c in range(CH):
                # bounce alpha + erbf of this chunk to DRAM
                nc.sync.dma_start(
                    AP(tensor=al2d, offset=c * TC * P * 256,
                       ap=[[256, P], [P * 256, TC], [1, 256]]),
                    R(al2, c, 256, [[256, TC], [1, 256]]),
                )
                nc.sync.dma_start(
                    AP(tensor=erbd, offset=c * TC * P * 96,
                       ap=[[96, P], [P * 96, TC], [1, 96]]),
                    R(erbf, c, 96, [[96, TC], [1, 96]]),
                )
                # moving operand: erbB[(jsub,b), (g,r)] <- erbd
                nc.sync.dma_start(
                    sub(erbB[:], c * GPC * 6, [[6, GPC], [1, 6]]),
                    AP(tensor=erbd, offset=c * GPC * 768,
                       ap=[[6, P], [768, GPC], [1, 6]]),
                )
                for rnd in range(GPC // RND):
                    w0 = c * GPC + rnd * RND
                    h = (w0 // RND) % 2
                    for js in range(8):
                        base = w2[js * 16 : (js + 1) * 16]
                        nc.sync.dma_start(
                            AP(tensor=base.tensor,
                               offset=base.offset + h * RND * P + js * 16,
                               ap=[list(base.ap[0]), [128, RND], [1, 16]]),
                            AP(tensor=al2d, offset=(w0 * 8 + js) * 256,
                               ap=[[16, 16], [2048, RND], [1, 16]]),
                        )
                    for g in range(RND):
                        nc.tensor.matmul(
                            sub(ps[:], (w0 + g) * 8, [[1, 6]]),
                            sub(w2[:], h * RND * P + g * P, [[1, P]]),
                            sub(erbB[:], (w0 + g) * 6, [[1, 6]]),
                            start=True, stop=True,
                        )
                # drain with x32 rescale, then store
                nc.scalar.activation(
                    R(ot, c, 96, [[6, GPC], [1, 6]]),
                    AP(tensor=ps.tensor, offset=ps[:].offset + c * GPC * 8,
                       ap=[list(ps[:].ap[0]), [8, GPC], [1, 6]]),
                    ACT.Copy, scale=32.0,
                )
                nc.sync.dma_start(
                    AP(tensor=out, offset=c * GPC * 768,
                       ap=[[6, P], [768, GPC], [1, 6]]),
                    R(ot, c, 96, [[6, GPC], [1, 6]]),
                )

    nc.compile()
    return nc


def _get_nc():
    global _cached_nc
    if _cached_nc is None:
        _cached_nc = _build_device_kernel()
    return _cached_nc


def _make_consts():
    ncv = np.zeros(16, np.float32)
    ncv[:6] = (np.arange(1, N_RBF + 1) / (2.0 * CUTOFF)).astype(np.float32)
    ncv[7] = -1.0
    return np.broadcast_to(ncv, (P, 16)).copy()


def _run_device(xyz, trace=False, tmpdir=None):
    from concourse import bass_utils

    nc = _get_nc()
    consts = _make_consts()
    ext = np.concatenate([xyz[-HALF:], xyz, xyz[:HALF]], axis=0)  # halo-extended
    in_maps = []
    for c in range(N_CORES):
        base = c * J_PER_CORE
        winc = np.ascontiguousarray(ext[base : base + WIN_ROWS])
        in_maps.append({"win": winc, "consts": consts})
    kwargs = {}
    if trace:
        kwargs = dict(trace=True)
        if tmpdir is not None:
            kwargs["tmpdir"] = tmpdir
    res = bass_utils.run_bass_kernel_spmd(
        nc, in_maps, core_ids=list(range(N_CORES)), **kwargs
    )
    shards = [res.results[c]["out"] for c in range(N_CORES)]
    full = np.concatenate(shards, axis=0).astype(np.float32)
    return full, res


def kernel(xyz, nbr_list, angle_list):
    xyz = np.asarray(xyz, dtype=np.float32)
    if not _graph_matches(nbr_list, angle_list):
        return _fallback_numpy(xyz, nbr_list, angle_list)
    out, _ = _run_device(xyz)
    return out


# revision 14
# speedup vs baseline: 1.7282x; 1.7282x over previous
"""Bass/Trainium2 kernel for nn_DimeNet_22737556865501.

Strategy (v2)
-------------
Same circulant-structure collapse as v1 (per-atom dense math on the 16
local displacement vectors), but restructured around three observations:

1. Normalize first: with Vhat = V/|V|, the Gram matrix of Vhat IS cos(alpha),
   eliminating the ab/amg/den/ln/exp chain entirely.
2. Half-angle: alpha = 2*atan(sqrt((1-c)/(1+c))) and
   (1-c)/(1+c) = 2/(1+c) - 1, so alpha costs exactly three ACT ops
   (reciprocal with bias, sqrt with scale/bias, arctan) after one clamp.
   The factor 2 folds into env2 = 2*env (already needed).
3. bf16 + DVE 2x packing for the two dense blocks (Gram pair products and
   the alpha@erbf contraction), with the c-dimension padded to 4 so the
   innermost axis stays even/step-1; reductions use DVE tensor_reduce or
   short trees with f32 final accumulation.

Activation tables used (one load each, phases globally ordered):
  reciprocal_sqrt_and_small (Square, Rsqrt) -> reciprocal_and_small
  (Reciprocal) -> sqrt_and_others (Sqrt) -> trig_and_small (Sin, Arctan).

Sharding: atoms partitioned across the 8 NeuronCores (4096 each); each core
writes its own 65536x6 output rows; host concatenates. Host verifies the
circulant graph and falls back to exact numpy otherwise.
"""

import numpy as np

N_ATOMS = 32768
DEG = 16
HALF = DEG // 2
N_CORES = 8
J_PER_CORE = N_ATOMS // N_CORES  # 4096
P = 128  # partitions / atoms per tile
N_TILES = J_PER_CORE // P  # 32
WIN_ROWS = J_PER_CORE + DEG  # 4112 (8-row halo each side)
N_RBF = 6
CUTOFF = 5.0
ENV_P = 6
A_ = -(ENV_P + 1) * (ENV_P + 2) / 2.0  # -28
B_ = float(ENV_P * (ENV_P + 2))  # 48
C_ = -ENV_P * (ENV_P + 1) / 2.0  # -21
EA = 2.0 * A_  # -56
EB = 2.0 * B_  # 96
EC = 2.0 * C_  # -42
TWO_PI = float(2.0 * np.pi)

# tile ownership: t % 8 < GD_DVE -> Gram on DVE; t % 8 < CD_DVE -> contraction
# on DVE (rest on Pool/GpSimd)
GD_DVE = 5
CD_DVE = 7

OFFS = np.concatenate([np.arange(1, HALF + 1), -np.arange(1, HALF + 1)])

_cached_nc = None


def _expected_graph():
    half = HALF
    offs = np.concatenate([np.arange(1, half + 1), N_ATOMS - np.arange(1, half + 1)])
    j = np.arange(N_ATOMS)
    nbr_dst = (j[:, None] + offs[None, :]) % N_ATOMS
    nbr_list = np.stack([np.repeat(j, DEG), nbr_dst.reshape(-1)], 1)
    o1, o2 = np.meshgrid(offs, offs, indexing="ij")
    keep = o1 != o2
    o1, o2 = o1[keep], o2[keep]
    i = (j[:, None] + o1[None, :]) % N_ATOMS
    k = (j[:, None] + o2[None, :]) % N_ATOMS
    jc = np.broadcast_to(j[:, None], i.shape)
    angle_list = np.stack([i.reshape(-1), jc.reshape(-1), k.reshape(-1)], 1)
    return nbr_list.astype(np.int64), angle_list.astype(np.int64)


def _graph_matches(nbr_list, angle_list):
    if nbr_list.shape != (N_ATOMS * DEG, 2):
        return False
    if angle_list.shape != (N_ATOMS * DEG * (DEG - 1), 3):
        return False
    exp_nbr, exp_ang = _expected_graph()
    return np.array_equal(np.asarray(nbr_list), exp_nbr) and np.array_equal(
        np.asarray(angle_list), exp_ang
    )


def _fallback_numpy(xyz, nbr_list, angle_list):
    """Exact numpy mirror of the jax reference (general graph)."""
    xyz = np.asarray(xyz, dtype=np.float32)
    nbr = np.asarray(nbr_list)
    ang = np.asarray(angle_list)
    E = nbr.shape[0]
    r_ji = xyz[ang[:, 0]] - xyz[ang[:, 1]]
    r_jk = xyz[ang[:, 2]] - xyz[ang[:, 1]]
    dot = np.sum(r_ji * r_jk, axis=-1)
    crs = np.linalg.norm(np.cross(r_ji, r_jk), axis=-1)
    alpha = np.arctan2(crs, dot)
    diff = xyz[nbr[:, 0]] - xyz[nbr[:, 1]]
    d = np.linalg.norm(diff, axis=-1)
    n = np.arange(1, N_RBF + 1, dtype=xyz.dtype)
    dc = (d / CUTOFF)[:, None]
    env = 1.0 / dc + A_ * dc ** (ENV_P - 1) + B_ * dc**ENV_P + C_ * dc ** (ENV_P + 1)
    e_rbf = env * np.sin(n * np.pi * dc)
    keys = nbr[:, 0] * N_ATOMS + nbr[:, 1]
    order = np.argsort(keys, kind="stable")
    ji_idx = order[np.searchsorted(keys[order], ang[:, 1] * N_ATOMS + ang[:, 0])]
    kj_idx = order[np.searchsorted(keys[order], ang[:, 2] * N_ATOMS + ang[:, 1])]
    trip = alpha[:, None] * e_rbf[kj_idx]
    out = np.zeros((E, N_RBF), dtype=np.float32)
    np.add.at(out, ji_idx, trip.astype(np.float32))
    return out


# ---------------------------------------------------------------------------
# Device kernel
# ---------------------------------------------------------------------------


def _build_device_kernel():
    import concourse.bacc as bacc
    import concourse.mybir as mybir
    from concourse.bass_types import AP
    from concourse.tile import TileContext

    F32 = mybir.dt.float32
    BF16 = mybir.dt.bfloat16
    I32 = mybir.dt.int32
    ALU = mybir.AluOpType
    ACT = mybir.ActivationFunctionType
    AX = mybir.AxisListType

    # Steer the activation-table-load pass so each function resolves to one
    # set and the phase ordering needs exactly four table loads.
    from concourse.hw_specs import get_activation_tables

    assign = {
        ACT.Square: "natural_log_exp_and_others",
        ACT.Ln: "natural_log_exp_and_others",
        ACT.Exp: "natural_log_exp_and_others",
        ACT.Sqrt: "sqrt_and_others",
        ACT.Sin: "trig_and_small",
        ACT.Arctan: "trig_and_small",
    }
    tabs = get_activation_tables("gen3")
    for name, fns in tabs.items():
        for fn, keep in assign.items():
            if name != keep:
                fns.discard(fn)

    def sub(base: AP, off: int, dims) -> AP:
        """Sub-AP of an SBUF tile: keep partition dim, custom free dims."""
        return AP(
            tensor=base.tensor,
            offset=base.offset + off,
            ap=[list(base.ap[0]), *[list(d) for d in dims]],
        )

    T = N_TILES  # 32
    CH = 4  # pipeline chunks
    TC = T // CH  # tiles per chunk
    nc = bacc.Bacc("TRN2", target_bir_lowering=False, debug=False, num_devices=N_CORES)
    win = nc.dram_tensor("win", [WIN_ROWS, 3], F32, kind="ExternalInput")
    consts = nc.dram_tensor("consts", [P, 16], F32, kind="ExternalInput")
    out = nc.dram_tensor("out", [J_PER_CORE * DEG, N_RBF], F32, kind="ExternalOutput")

    with TileContext(nc) as tc:
        with (
            tc.tile_pool(name="big", bufs=1) as big,
            tc.tile_pool(name="work", bufs=2) as work,
        ):
            nco = big.tile([P, 16], F32, name="nco")
            nc.sync.dma_start(nco[:], consts[:])
            # register -1.0 (held in consts slot 7) for activation bias use
            nc.const_aps.aps[(F32, -1.0)] = sub(nco[:], 7, [[1, 1]])

            # ---- global buffers (per-partition free sizes) ----
            w = big.tile([P, T * 51], F32, name="w")  # window
            v = big.tile([P, T * 48], F32, name="v")  # V f32 [t,b,3]
            n2 = big.tile([P, T * 16], F32, name="n2")
            yn = big.tile([P, T * 16], F32, name="yn")  # 1/d
            dd = big.tile([P, T * 16], F32, name="dd")  # d
            vh = big.tile([P, T * 48], F32, name="vh")  # Vhat f32 [t,b,3]
            dc = big.tile([P, T * 16], F32, name="dc")
            q = big.tile([P, T * 16], F32, name="q")
            x5 = big.tile([P, T * 16], F32, name="x5")
            h1 = big.tile([P, T * 16], F32, name="h1")
            env2 = big.tile([P, T * 16], F32, name="env2")
            sa2 = big.tile([P, T * 96], F32, name="sa2")  # [t,r,b] turns
            ki = big.tile([P, T * 96], I32, name="ki")
            kf = big.tile([P, T * 96], F32, name="kf")
            sinv = big.tile([P, T * 96], F32, name="sinv")
            erbf = big.tile([P, T * 96], BF16, name="erbf")  # [t,r,b]
            ch = big.tile([P, T * 256], F32, name="ch")  # cos alpha [t,b,a]
            rec = big.tile([P, T * 256], F32, name="rec")
            al2 = big.tile([P, T * 256], BF16, name="al2")  # alpha/2 bf16
            ot = big.tile([P, T * 96], F32, name="ot")  # out [t,a,r]

            def R(buf, c, per, dims):
                return sub(buf[:], c * TC * per, dims)

            # ---------- phase-major, chunk-minor pipeline ----------
            # window loads (one DMA per chunk)
            for c in range(CH):
                src = AP(
                    tensor=win,
                    offset=c * TC * P * 3,
                    ap=[[3, P], [P * 3, TC], [1, 51]],
                )
                nc.sync.dma_start(R(w, c, 51, [[1, TC * 51]]), src)

            # V[t,a,c]; a=0..7 <- +1..+8, a=8..15 <- -1..-8  (Pool)
            for c in range(CH):
                ctr = R(w, c, 51, [[51, TC], [0, 8], [1, 3]])
                ctr = AP(tensor=ctr.tensor, offset=ctr.offset + 24, ap=ctr.ap)
                nc.gpsimd.tensor_tensor(
                    R(v, c, 48, [[48, TC], [3, 8], [1, 3]]),
                    sub(R(w, c, 51, [[51, TC], [3, 8], [1, 3]]), 27, None)
                    if False
                    else AP(
                        tensor=w.tensor,
                        offset=w[:].offset + c * TC * 51 + 27,
                        ap=[list(w[:].ap[0]), [51, TC], [3, 8], [1, 3]],
                    ),
                    ctr,
                    ALU.subtract,
                )
                nc.gpsimd.tensor_tensor(
                    AP(
                        tensor=v.tensor,
                        offset=v[:].offset + c * TC * 48 + 24,
                        ap=[list(v[:].ap[0]), [48, TC], [3, 8], [1, 3]],
                    ),
                    AP(
                        tensor=w.tensor,
                        offset=w[:].offset + c * TC * 51 + 21,
                        ap=[list(w[:].ap[0]), [51, TC], [-3, 8], [1, 3]],
                    ),
                    ctr,
                    ALU.subtract,
                )

            # ---- norms: n2 = sum_c V^2 (Pool, 5 ops using q as scratch) ----
            for c in range(CH):
                vv0 = R(v, c, 48, [[48, TC], [3, 16], [0, 1]])
                vv1 = AP(tensor=v.tensor, offset=v[:].offset + c * TC * 48 + 1,
                         ap=[list(v[:].ap[0]), [48, TC], [3, 16], [0, 1]])
                vv2 = AP(tensor=v.tensor, offset=v[:].offset + c * TC * 48 + 2,
                         ap=[list(v[:].ap[0]), [48, TC], [3, 16], [0, 1]])
                n2c = R(n2, c, 16, [[16, TC], [1, 16], [0, 1]])
                qc = R(q, c, 16, [[16, TC], [1, 16], [0, 1]])
                nc.gpsimd.tensor_tensor(n2c, vv0, vv0, ALU.mult)
                nc.gpsimd.tensor_tensor(qc, vv1, vv1, ALU.mult)
                nc.gpsimd.tensor_tensor(n2c, n2c, qc, ALU.add)
                nc.gpsimd.tensor_tensor(qc, vv2, vv2, ALU.mult)
                nc.gpsimd.tensor_tensor(n2c, n2c, qc, ALU.add)

            # ---- 1/d via exp(-0.5 ln n2) (ACT, natural_log_exp set) ----
            for c in range(CH):
                nc.scalar.activation(
                    R(yn, c, 16, [[1, TC * 16]]), R(n2, c, 16, [[1, TC * 16]]),
                    ACT.Ln,
                )
                nc.scalar.activation(
                    R(yn, c, 16, [[1, TC * 16]]), R(yn, c, 16, [[1, TC * 16]]),
                    ACT.Exp, scale=-0.5,
                )

            for c in range(CH):
                # d = n2 * yn ; Vhat = V * yn (DVE)
                nc.vector.tensor_tensor(
                    R(dd, c, 16, [[1, TC * 16]]),
                    R(n2, c, 16, [[1, TC * 16]]),
                    R(yn, c, 16, [[1, TC * 16]]),
                    ALU.mult,
                )
                nc.vector.tensor_tensor(
                    R(vh, c, 48, [[48, TC], [3, 16], [1, 3]]),
                    R(v, c, 48, [[48, TC], [3, 16], [1, 3]]),
                    R(yn, c, 16, [[16, TC], [1, 16], [0, 3]]),
                    ALU.mult,
                )

            # ---- envelope: env2 = 2C*yn + dc^5*(EA + EB*dc + EC*dc^2) ----
            for c in range(CH):
                dcc = R(dc, c, 16, [[1, TC * 16]])
                qc = R(q, c, 16, [[1, TC * 16]])
                x5c = R(x5, c, 16, [[1, TC * 16]])
                h1c = R(h1, c, 16, [[1, TC * 16]])
                nc.vector.tensor_scalar(
                    dcc, R(dd, c, 16, [[1, TC * 16]]), 1.0 / CUTOFF, None, ALU.mult
                )
                nc.gpsimd.tensor_tensor(qc, dcc, dcc, ALU.mult)
                nc.gpsimd.tensor_tensor(x5c, qc, qc, ALU.mult)
                nc.gpsimd.tensor_tensor(x5c, x5c, dcc, ALU.mult)
                nc.vector.tensor_scalar(h1c, dcc, EC, EB, ALU.mult, ALU.add)
                nc.gpsimd.tensor_tensor(h1c, h1c, dcc, ALU.mult)
                nc.vector.tensor_scalar(h1c, h1c, EA, None, ALU.add)
                nc.gpsimd.tensor_tensor(x5c, x5c, h1c, ALU.mult)
                nc.vector.scalar_tensor_tensor(
                    R(env2, c, 16, [[1, TC * 16]]),
                    R(yn, c, 16, [[1, TC * 16]]),
                    2.0 * CUTOFF, x5c, ALU.mult, ALU.add,
                )

            # ---- sin args (turns) + range reduction ----
            for c in range(CH):
                nc.vector.tensor_tensor(
                    R(sa2, c, 96, [[96, TC], [16, 6], [1, 16]]),
                    R(dd, c, 16, [[16, TC], [0, 6], [1, 16]]),
                    sub(nco[:], 0, [[0, TC], [1, 6], [0, 16]]),
                    ALU.mult,
                )
                nc.vector.tensor_copy(
                    R(ki, c, 96, [[1, TC * 96]]), R(sa2, c, 96, [[1, TC * 96]])
                )
                nc.vector.tensor_copy(
                    R(kf, c, 96, [[1, TC * 96]]), R(ki, c, 96, [[1, TC * 96]])
                )
                nc.gpsimd.tensor_tensor(
                    R(sa2, c, 96, [[1, TC * 96]]),
                    R(sa2, c, 96, [[1, TC * 96]]),
                    R(kf, c, 96, [[1, TC * 96]]),
                    ALU.subtract,
                )

            # ---- Gram (Pool, f32): ch[t,b,a] = sum_c Vhat[b,c]*Vhat[a,c] ----
            for c in range(CH):
                for tl in range(TC):
                    t = c * TC + tl
                    p3 = work.tile([P, 768], F32, tag="p3", bufs=2)
                    nc.gpsimd.tensor_tensor(
                        sub(p3[:], 0, [[48, 16], [3, 16], [1, 3]]),
                        sub(vh[:], t * 48, [[0, 16], [3, 16], [1, 3]]),
                        sub(vh[:], t * 48, [[3, 16], [0, 16], [1, 3]]),
                        ALU.mult,
                    )
                    nc.gpsimd.tensor_tensor(
                        sub(ch[:], t * 256, [[1, 256]]),
                        sub(p3[:], 0, [[3, 256]]),
                        sub(p3[:], 1, [[3, 256]]),
                        ALU.add,
                    )
                    nc.gpsimd.tensor_tensor(
                        sub(ch[:], t * 256, [[1, 256]]),
                        sub(ch[:], t * 256, [[1, 256]]),
                        sub(p3[:], 2, [[3, 256]]),
                        ALU.add,
                    )

            # clamp so both ln args stay positive: |c| <= 1 - 2^-23
            for c in range(CH):
                nc.vector.tensor_scalar(
                    R(ch, c, 256, [[1, TC * 256]]),
                    R(ch, c, 256, [[1, TC * 256]]),
                    0.9999999, -0.9999999, ALU.min, ALU.max,
                )

            # ---- alpha/2 = atan(exp(0.5*(ln(1-c) - ln(1+c)))) ----
            for c in range(CH):
                cc = R(ch, c, 256, [[1, TC * 256]])
                rc = R(rec, c, 256, [[1, TC * 256]])
                nc.scalar.activation(rc, cc, ACT.Ln, bias=1.0)  # ln(1+c)
                nc.scalar.activation(cc, cc, ACT.Ln, bias=1.0, scale=-1.0)
                nc.vector.tensor_tensor(cc, cc, rc, ALU.subtract)
                nc.scalar.activation(rc, cc, ACT.Exp, scale=0.5)
            # sin goes through the same-set boundary; trig set loads once
            for c in range(CH):
                nc.scalar.activation(
                    R(al2, c, 256, [[1, TC * 256]]),
                    R(rec, c, 256, [[1, TC * 256]]),
                    ACT.Arctan,
                )
                nc.scalar.activation(
                    R(sinv, c, 96, [[1, TC * 96]]),
                    R(sa2, c, 96, [[1, TC * 96]]),
                    ACT.Sin, scale=TWO_PI,
                )
            # diagonal alpha := 0 (reference's i != k exclusion)
            for c in range(CH):
                dg = AP(
                    tensor=al2.tensor,
                    offset=al2[:].offset + c * TC * 256,
                    ap=[list(al2[:].ap[0]), [256, TC], [17, 16]],
                )
                nc.vector.tensor_scalar_mul(dg, dg, 0.0)

            # ---- e_rbf (x2): erbf[t,r,b] = env2[t,b] * sinv[t,r,b] (Pool) ----
            for c in range(CH):
                nc.gpsimd.tensor_tensor(
                    R(erbf, c, 96, [[96, TC], [16, 6], [1, 16]]),
                    R(sinv, c, 96, [[96, TC], [16, 6], [1, 16]]),
                    R(env2, c, 16, [[16, TC], [0, 6], [1, 16]]),
                    ALU.mult,
                )

            # ---- contraction (DVE): ot[t,a,r] = sum_b al2[t,a,b]*erbf[t,r,b]
            for c in range(CH):
                for tl in range(TC):
                    t = c * TC + tl
                    p4 = work.tile([P, 1536], BF16, tag="p4", bufs=2)
                    t1 = work.tile([P, 768], BF16, tag="t1", bufs=2)
                    t2 = work.tile([P, 384], BF16, tag="t2", bufs=2)
                    t3 = work.tile([P, 192], F32, tag="t3", bufs=2)
                    nc.vector.tensor_tensor(
                        sub(p4[:], 0, [[96, 16], [16, 6], [1, 16]]),
                        sub(al2[:], t * 256, [[16, 16], [0, 6], [1, 16]]),
                        sub(erbf[:], t * 96, [[0, 16], [16, 6], [1, 16]]),
                        ALU.mult,
                    )
                    nc.vector.tensor_tensor(
                        sub(t1[:], 0, [[8, 96], [1, 8]]),
                        sub(p4[:], 0, [[16, 96], [1, 8]]),
                        sub(p4[:], 8, [[16, 96], [1, 8]]),
                        ALU.add,
                    )
                    nc.vector.tensor_tensor(
                        sub(t2[:], 0, [[4, 96], [1, 4]]),
                        sub(t1[:], 0, [[8, 96], [1, 4]]),
                        sub(t1[:], 4, [[8, 96], [1, 4]]),
                        ALU.add,
                    )
                    nc.vector.tensor_tensor(
                        sub(t3[:], 0, [[2, 96], [1, 2]]),
                        sub(t2[:], 0, [[4, 96], [1, 2]]),
                        sub(t2[:], 2, [[4, 96], [1, 2]]),
                        ALU.add,
                    )
                    nc.vector.tensor_tensor(
                        sub(ot[:], t * 96, [[1, 96]]),
                        sub(t3[:], 0, [[2, 96]]),
                        sub(t3[:], 1, [[2, 96]]),
                        ALU.add,
                    )
                # out rows (t*128 + p)*16 + a, cols r  (one DMA per chunk)
                dst = AP(
                    tensor=out,
                    offset=c * TC * P * 96,
                    ap=[[96, P], [96 * P, TC], [1, 96]],
                )
                nc.sync.dma_start(dst, R(ot, c, 96, [[96, TC], [1, 96]]))

    nc.compile()
    return nc


def _get_nc():
    global _cached_nc
    if _cached_nc is None:
        _cached_nc = _build_device_kernel()
    return _cached_nc


def _make_consts():
    ncv = np.zeros(16, np.float32)
    ncv[:6] = (np.arange(1, N_RBF + 1) / (2.0 * CUTOFF)).astype(np.float32)
    ncv[7] = -1.0
    return np.broadcast_to(ncv, (P, 16)).copy()


def _run_device(xyz, trace=False, tmpdir=None):
    from concourse import bass_utils

    nc = _get_nc()
    consts = _make_consts()
    ext = np.concatenate([xyz[-HALF:], xyz, xyz[:HALF]], axis=0)  # halo-extended
    in_maps = []
    for c in range(N_CORES):
        base = c * J_PER_CORE
        winc = np.ascontiguousarray(ext[base : base + WIN_ROWS])
        in_maps.append({"win": winc, "consts": consts})
    kwargs = {}
    if trace:
        kwargs = dict(trace=True)
        if tmpdir is not None:
            kwargs["tmpdir"] = tmpdir
    res = bass_utils.run_bass_kernel_spmd(
        nc, in_maps, core_ids=list(range(N_CORES)), **kwargs
    )
    shards = [res.results[c]["out"] for c in range(N_CORES)]
    full = np.concatenate(shards, axis=0).astype(np.float32)
    return full, res


def kernel(xyz, nbr_list, angle_list):
    xyz = np.asarray(xyz, dtype=np.float32)
    if not _graph_matches(nbr_list, angle_list):
        return _fallback_numpy(xyz, nbr_list, angle_list)
    out, _ = _run_device(xyz)
    return out
